# revision 1
# baseline (speedup 1.0000x reference)
"""AttentiveGraphConvolution (GAT-style layer) on 8 trn2 NeuronCores.

Math (reference):
    h   = x @ W                       [N, D]
    a_s = h @ attn_self               [N, 1]
    a_n = h @ attn_neigh              [N, 1]
    e   = leaky_relu(a_s + a_n.T, 0.2)
    e   = e + NEG_INF * (1 - adj)
    out = relu(softmax(e, -1) @ h)

Reformulation used here (exact in fp32 up to rounding):
    s_ij = a_s[i] + a_n[j]
    exp(leaky(s)) = exp(0.2 s) * max(exp(0.8 s), 1)       (leaky alpha = 0.2)
    exp(0.8 s)    = w[i] * w2[j],  w = e^{0.8 a_s}, w2 = e^{0.8 a_n}
    adj binary =>  masked weight t_ij = adj_ij * u2_i * v2_j * max(w_i w2_j, 1)

    out_i = relu( (sum_j t_ij h_j) / (sum_j t_ij) )
          = relu( (sum_j q_ji h2_j) / (sum_j q_ji v2_j) )   (u2_i cancels)
    with q_ji  = adjT_ji * max(w_i w2_j, 1)                 [j, i] layout
         h2_j  = v2_j * h_j

Per adj tile the device work is:  R = w2_j * W_bcast  (ACT copy-with-scale),
q = (R max 1) * adjT  (DVE scalar_tensor_tensor), then accumulating float32r
matmuls  outT += h2_chunk.T @ q  and  rs += v2_chunk.T @ q  on the PE.

Sharding: output rows across 8 cores. Each core receives its adj row-slab as
bf16 (adj is binary so bf16 is exact), pre-transposed and row-interleaved in
groups of GP=4 (host layout choice) so each DMA descriptor covers 4 adjacency
rows = 8 KB contiguous.  x is sharded; h2 shards are computed locally,
written in a partition-major layout, and AllGathered (~4 MB) through DRAM so
the read-back also gets 4 KB descriptors.
"""

import numpy as np

N = 8192
DIN = 512
DOUT = 128
NCORES = 8
S = N // NCORES     # 1024 output rows per core
GP = 4              # adjacency rows per partition per DMA (descriptor size)


def _emit(nc, tc, ctx, n, s, din, dout):
    from concourse import masks, mybir

    f32 = mybir.dt.float32
    f32r = mybir.dt.float32r
    bf16 = mybir.dt.bfloat16
    AF = mybir.ActivationFunctionType
    ALU = mybir.AluOpType

    P = 128
    jc_n = n // P       # j chunks over all nodes
    sc_n = s // P       # chunks in the local row slab
    kc_n = din // P     # contraction chunks for x @ W
    nb = min(512, s)    # matmul moving-dim block
    ib_n = s // nb      # i blocks per core (free dim of main matmuls)
    g_n = jc_n // GP    # adj super-chunks (GP j-chunks per DMA)

    adjt = nc.dram_tensor("adjt", [n, s], bf16, kind="ExternalInput")
    xt = nc.dram_tensor("xt", [din, s], f32r, kind="ExternalInput")
    wmat = nc.dram_tensor("wmat", [din, dout], f32r, kind="ExternalInput")
    att = nc.dram_tensor("att", [dout, 2], f32, kind="ExternalInput")
    out = nc.dram_tensor("out", [s, dout], f32, kind="ExternalOutput")

    const_pool = ctx.enter_context(tc.tile_pool(name="const", bufs=1))
    ph1_pool = ctx.enter_context(tc.tile_pool(name="ph1", bufs=1))
    ph1_psum = ctx.enter_context(tc.tile_pool(name="ph1_psum", bufs=1, space="PSUM"))
    tp_psum = ctx.enter_context(tc.tile_pool(name="tp_psum", bufs=2, space="PSUM"))
    acc_psum = ctx.enter_context(tc.tile_pool(name="acc_psum", bufs=1, space="PSUM"))
    dram_pool = ctx.enter_context(tc.tile_pool(name="dram", bufs=1, space="DRAM"))
    adj_pool = ctx.enter_context(tc.tile_pool(name="adj", bufs=6))
    r_pool = ctx.enter_context(tc.tile_pool(name="r", bufs=5))
    q_pool = ctx.enter_context(tc.tile_pool(name="q", bufs=8))
    fin_pool = ctx.enter_context(tc.tile_pool(name="fin", bufs=2))

    ident = const_pool.tile([P, P], f32, name="ident")
    masks.make_identity(nc, ident[:])

    # ---- Phase 1: local h shard, attention logit vectors -------------------
    w_sb = []
    x_sb = []
    for k in range(kc_n):
        wt = ph1_pool.tile([P, P], f32r, name="w_sb", tag=f"w_sb{k}")
        nc.sync.dma_start(wt[:], wmat[k * P:(k + 1) * P, :])
        w_sb.append(wt)
        xt_t = ph1_pool.tile([P, s], f32r, name="x_sb", tag=f"x_sb{k}")
        nc.sync.dma_start(xt_t[:], xt[k * P:(k + 1) * P, :])
        x_sb.append(xt_t)
    att_sb = const_pool.tile([P, 2], f32, name="att_sb")
    nc.sync.dma_start(att_sb[:], att[:])

    # hT[d, n_local] = (x @ W).T for the local slab, built nb columns at a time
    hT_sb = ph1_pool.tile([P, s], f32, name="hT_sb")
    av_sb = ph1_pool.tile([2, s], f32, name="av_sb")  # rows: a_s, a_n (local)
    for b in range(s // nb):
        hT_ps = ph1_psum.tile([P, nb], f32, name="hT_ps")
        for k in range(kc_n):
            nc.tensor.matmul(
                hT_ps[:],
                w_sb[k][:],
                x_sb[k][:, b * nb:(b + 1) * nb],
                start=(k == 0),
                stop=(k == kc_n - 1),
            )
        nc.scalar.activation(hT_sb[:, b * nb:(b + 1) * nb], hT_ps[:], AF.Copy)
        av_ps = ph1_psum.tile([2, nb], f32, name="av_ps")
        nc.tensor.matmul(
            av_ps[:], att_sb[:], hT_sb[:, b * nb:(b + 1) * nb],
            start=True, stop=True,
        )
        nc.scalar.activation(av_sb[:, b * nb:(b + 1) * nb], av_ps[:], AF.Copy)

    # ---- Phase 2a: gather raw a_n early (small, unblocks the main loop) ----
    groups = [list(range(NCORES))]
    an_dram = dram_pool.tile([s, 1], f32, name="an_dram")
    nc.sync.dma_start(an_dram[:].rearrange("s o -> o s"), av_sb[1:2, :])
    anfull_dram = dram_pool.tile([n, 1], f32, addr_space="Shared", name="anfull")
    nc.gpsimd.collective_compute(
        "AllGather", ALU.bypass, replica_groups=groups,
        ins=[an_dram.opt()], outs=[anfull_dram.opt()],
    )
    anf_raw = ph1_pool.tile([jc_n, P], f32, name="anf_raw")
    nc.sync.dma_start(anf_raw[:], anfull_dram[:].rearrange("(k p) o -> k (p o)", p=P))
    anf_ps = tp_psum.tile([P, jc_n], f32, name="anf_ps", tag="tp")
    nc.tensor.matmul(anf_ps[:], anf_raw[:], ident[:jc_n, :jc_n],
                     is_transpose=True, start=True, stop=True)
    w2_sb = const_pool.tile([P, jc_n], f32, name="w2_sb")
    nc.scalar.activation(w2_sb[:], anf_ps[:], AF.Exp, scale=0.8)
    v2f_sb = const_pool.tile([P, jc_n], f32r, name="v2f_sb")
    nc.scalar.activation(v2f_sb[:], anf_ps[:], AF.Exp, scale=0.2)

    # W_bcast[p, i] = exp(0.8 * a_s_local[i]) for every partition p
    wrow_sb = ph1_pool.tile([1, s], f32, name="wrow_sb")
    nc.scalar.activation(wrow_sb[:], av_sb[0:1, :], AF.Exp, scale=0.8)
    ones_sb = const_pool.tile([1, P], f32, name="ones_sb")
    nc.gpsimd.memset(ones_sb[:], 1.0)
    wb_sb = const_pool.tile([P, s], f32, name="wb_sb")
    for b in range(s // nb):
        wb_ps = tp_psum.tile([P, nb], f32, name="wb_ps", tag="tp")
        nc.tensor.matmul(
            wb_ps[:], ones_sb[:], wrow_sb[:, b * nb:(b + 1) * nb],
            start=True, stop=True,
        )
        nc.scalar.activation(wb_sb[:, b * nb:(b + 1) * nb], wb_ps[:], AF.Copy)

    # ---- Phase 2b: h2 shard in partition-major layout, AllGather -----------
    # Local chunk c is written to rows {p*sc_n + c} so that the gathered
    # tensor reads back with 4 KB-contiguous per-partition descriptors.
    anT_sb = ph1_pool.tile([P, sc_n], f32, name="anT_sb")
    for c in range(sc_n):
        avT_ps = tp_psum.tile([P, 2], f32, name="avT_ps", tag="tp")
        nc.tensor.matmul(
            avT_ps[:], av_sb[:, c * P:(c + 1) * P], ident[:2, :2],
            is_transpose=True, start=True, stop=True,
        )
        nc.scalar.activation(anT_sb[:, c:c + 1], avT_ps[:, 1:2], AF.Copy)
    v2loc_sb = ph1_pool.tile([P, sc_n], f32, name="v2loc_sb")
    nc.scalar.activation(v2loc_sb[:], anT_sb[:], AF.Exp, scale=0.2)

    h2an_dram = dram_pool.tile([s, dout], f32r, name="h2an_dram")
    h2an_pm = h2an_dram[:].rearrange("(p kl) d -> kl p d", kl=sc_n)
    for c in range(sc_n):
        hn_ps = tp_psum.tile([P, P], f32, name="hn_ps", tag="tp")
        nc.tensor.matmul(
            hn_ps[:], hT_sb[:, c * P:(c + 1) * P], ident[:],
            is_transpose=True, start=True, stop=True,
        )
        h2c_sb = fin_pool.tile([P, dout], f32r, name="h2c_sb")
        nc.scalar.activation(h2c_sb[:], hn_ps[:], AF.Copy, scale=v2loc_sb[:, c:c + 1])
        nc.sync.dma_start(h2an_pm[c], h2c_sb[:])

    h2full_dram = dram_pool.tile([n, dout], f32r, addr_space="Shared", name="h2full")
    nc.gpsimd.collective_compute(
        "AllGather", ALU.bypass, replica_groups=groups,
        ins=[h2an_dram.opt()], outs=[h2full_dram.opt()],
    )

    # ---- Phase 3: load gathered h2 (4 KB descriptors per core block) -------
    h2big = ph1_pool.tile([P, jc_n * dout], f32r, name="h2big")
    for c in range(NCORES):
        nc.sync.dma_start(
            h2big[:, c * sc_n * dout:(c + 1) * sc_n * dout],
            h2full_dram[c * s:(c + 1) * s, :].rearrange(
                "(p kl) d -> p (kl d)", kl=sc_n),
        )

    # ---- Phase 4: main loop over adj super-chunks --------------------------
    mm_ps = [acc_psum.tile([P, nb], f32, name=f"mm_ps{b}") for b in range(ib_n)]
    rs_ps = [acc_psum.tile([1, nb], f32, name=f"rs_ps{b}") for b in range(ib_n)]
    for g in range(g_n):
        adj_t = adj_pool.tile([P, GP * s], bf16, name="adj_t")
        nc.sync.dma_start(
            adj_t[:],
            adjt[g * GP * P:(g + 1) * GP * P, :].rearrange(
                "(p r) i -> p (r i)", r=GP),
        )
        for r in range(GP):
            j = g * GP + r
            r_t = r_pool.tile([P, s], f32, name="r_t")
            nc.scalar.activation(r_t[:], wb_sb[:], AF.Copy, scale=w2_sb[:, j:j + 1])
            q_t = q_pool.tile([P, s], f32r, name="q_t")
            nc.vector.scalar_tensor_tensor(
                q_t[:], r_t[:], 1.0, adj_t[:, r * s:(r + 1) * s],
                op0=ALU.max, op1=ALU.mult,
            )
            for b in range(ib_n):
                nc.tensor.matmul(
                    mm_ps[b][:], h2big[:, j * dout:(j + 1) * dout],
                    q_t[:, b * nb:(b + 1) * nb],
                    start=(j == 0), stop=(j == jc_n - 1),
                )
            for b in range(ib_n):
                nc.tensor.matmul(
                    rs_ps[b][:], v2f_sb[:, j:j + 1], q_t[:, b * nb:(b + 1) * nb],
                    start=(j == 0), stop=(j == jc_n - 1),
                )

    # ---- Phase 5: normalize, relu, transpose out ---------------------------
    rs_sb = ph1_pool.tile([1, s], f32, name="rs_sb")
    for b in range(ib_n):
        nc.scalar.activation(rs_sb[:, b * nb:(b + 1) * nb], rs_ps[b][:], AF.Copy)
    rs_dram = dram_pool.tile([sc_n, P], f32, name="rs_dram")
    nc.sync.dma_start(rs_dram[:].rearrange("k p -> (k p)")[None, :], rs_sb[0:1, :])
    rs_raw = ph1_pool.tile([sc_n, P], f32, name="rs_raw")
    nc.sync.dma_start(rs_raw[:], rs_dram[:])
    rsT_ps = tp_psum.tile([P, sc_n], f32, name="rsT_ps", tag="tp")
    nc.tensor.matmul(rsT_ps[:], rs_raw[:], ident[:sc_n, :sc_n],
                     is_transpose=True, start=True, stop=True)
    rrT_sb = ph1_pool.tile([P, sc_n], f32, name="rrT_sb")
    nc.vector.reciprocal(rrT_sb[:], rsT_ps[:])

    mo_sb = ph1_pool.tile([P, s], f32, name="mo_sb")
    for b in range(ib_n):
        nc.scalar.activation(mo_sb[:, b * nb:(b + 1) * nb], mm_ps[b][:], AF.Copy)
    for c in range(sc_n):
        ot_ps = tp_psum.tile([P, P], f32, name="ot_ps", tag="tp")
        nc.tensor.matmul(
            ot_ps[:], mo_sb[:, c * P:(c + 1) * P], ident[:],
            is_transpose=True, start=True, stop=True,
        )
        oc_sb = fin_pool.tile([P, dout], f32, name="oc_sb")
        nc.scalar.activation(oc_sb[:], ot_ps[:], AF.Relu, scale=rrT_sb[:, c:c + 1])
        nc.sync.dma_start(out[c * P:(c + 1) * P, :], oc_sb[:])


def build_nc(n=N, s=S, din=DIN, dout=DOUT):
    from contextlib import ExitStack

    import concourse.bacc as bacc
    import concourse.tile as tile

    nc = bacc.Bacc(
        "TRN2",
        target_bir_lowering=False,
        debug=False,
        num_devices=NCORES,
    )
    with tile.TileContext(nc) as tc, ExitStack() as ctx:
        _emit(nc, tc, ctx, n, s, din, dout)
    nc.compile()
    return nc


def prep_adjt(adj_slab):
    """[s, n] adj row-slab -> transposed [n, s] bf16 with GP-row interleave."""
    import ml_dtypes

    adjt = adj_slab.T  # [n, s]
    n, s = adjt.shape
    P = 128
    g = n // (GP * P)
    adjt = adjt.reshape(g, GP, P, s).transpose(0, 2, 1, 3).reshape(n, s)
    return np.ascontiguousarray(adjt.astype(ml_dtypes.bfloat16))


def make_in_maps(x, adj, W, attn_self, attn_neigh, s=S):
    att = np.concatenate([attn_self, attn_neigh], axis=1).astype(np.float32)
    in_maps = []
    for c in range(NCORES):
        sl = slice(c * s, (c + 1) * s)
        in_maps.append({
            "adjt": prep_adjt(adj[sl, :]),
            "xt": np.ascontiguousarray(x[sl, :].T),
            "wmat": np.ascontiguousarray(W),
            "att": att,
        })
    return in_maps


def kernel(x, adj, W, attn_self, attn_neigh):
    from concourse.bass_utils import run_bass_kernel_spmd

    x = np.asarray(x, dtype=np.float32)
    adj = np.asarray(adj, dtype=np.float32)
    W = np.asarray(W, dtype=np.float32)
    attn_self = np.asarray(attn_self, dtype=np.float32)
    attn_neigh = np.asarray(attn_neigh, dtype=np.float32)

    nc = build_nc()
    in_maps = make_in_maps(x, adj, W, attn_self, attn_neigh)
    res = run_bass_kernel_spmd(nc, in_maps, list(range(NCORES)))
    return np.concatenate([res.results[c]["out"] for c in range(NCORES)], axis=0)



# revision 17
# speedup vs baseline: 1.4541x; 1.4541x over previous
"""AttentiveGraphConvolution (GAT-style layer) on 8 trn2 NeuronCores.

Math (reference):
    h   = x @ W                       [N, D]
    a_s = h @ attn_self               [N, 1]
    a_n = h @ attn_neigh              [N, 1]
    e   = leaky_relu(a_s + a_n.T, 0.2)
    e   = e + NEG_INF * (1 - adj)
    out = relu(softmax(e, -1) @ h)

Reformulation (exact up to rounding), with s_ij = a_s[i] + a_n[j]:
    exp(leaky(s)) = exp(0.2 s) * max(exp(0.8 s), 1)
                  = u2_i * max(w_i * v_j, v2_j)
      w = e^{0.8 a_s},  v = e^{a_n},  v2 = e^{0.2 a_n}  (u2_i cancels in softmax)
    out_i = relu( (sum_j q_ji h3_j[:D]) / (sum_j q_ji h3_j[D]) )
      q_ji = adjT_ji * max(w_i * v_j, v2_j)     [j, i] layout (v2 folded in)
      h3_j = [h_j | 1]                          [j, D+1]  (denominator folded in)

Device work per 8-chunk group (1024 j's) per core:
    t_ji = max(w_i * v_j, v2_j)   8x DVE tensor_scalar (4x bf16 mode,
                                  per-partition AP scalars)
    q    = t * adjT               1x DVE tensor_tensor (2x bf16 mode)
    64x matmul: psum[i_blk, 0:130] += q[:, blk].T @ h3[j-chunk]  (q stationary)
The 130-wide moving operand folds the softmax denominator into the matmul
stream (col 128 = ones, col 129 = pad for 4-byte alignment); the output
lands directly in [i, d] layout; there is no second rs matmul pass.

NO COLLECTIVES: each core loads the full x (bf16, 8.4 MB) and computes the
full h3 locally -- profiling showed AllGather rendezvous + protocol cost
~70 us, far more than the extra DMA. Inputs are HOST-ROTATED per core
(node axis rolled so the core's own slab is block 0), which keeps the
program rank-independent: a_s is always read from columns [0, S).
"""

import numpy as np

N = 8192
DIN = 512
DOUT = 128
NCORES = 8
S = N // NCORES   # 1024 output rows per core
GP = 4            # adjacency j-chunks per DMA descriptor group (8 KB)
MG = 8            # j-chunks per merge group (DVE op granularity)
DEBUG = False     # add intermediate-dump outputs (set before build_nc)


def _emit(nc, tc, ctx, n, s, din, dout):
    from concourse import masks, mybir

    f32 = mybir.dt.float32
    bf16 = mybir.dt.bfloat16
    AF = mybir.ActivationFunctionType
    ALU = mybir.AluOpType

    P = 128
    jc_n = n // P        # 64 j chunks over all nodes
    kc_n = din // P      # 4 contraction chunks for x @ W
    nb = 512             # moving-dim block for phase-1 matmuls
    mg_n = jc_n // MG    # 8 merge groups
    ib_n = s // P        # 8 output row blocks
    dp = dout + 2        # h3 stride: [h | 1 | pad] -> 130 (4-byte aligned)
    XT = 2048            # x tile width (4 KB descriptors)

    adjt = nc.dram_tensor("adjt", [n, s], bf16, kind="ExternalInput")
    xt = nc.dram_tensor("xt", [din, n], bf16, kind="ExternalInput")
    wmat = nc.dram_tensor("wmat", [din, dout], bf16, kind="ExternalInput")
    att = nc.dram_tensor("att", [dout, 2], bf16, kind="ExternalInput")
    out = nc.dram_tensor("out", [s, dout], f32, kind="ExternalOutput")

    const_pool = ctx.enter_context(tc.tile_pool(name="const", bufs=1))
    ph1_pool = ctx.enter_context(tc.tile_pool(name="ph1", bufs=1))
    x_pool = ctx.enter_context(tc.tile_pool(name="x", bufs=5))
    tp_psum = ctx.enter_context(tc.tile_pool(name="tp_psum", bufs=3, space="PSUM"))
    acc_psum = ctx.enter_context(tc.tile_pool(name="acc_psum", bufs=1, space="PSUM"))
    dram_pool = ctx.enter_context(tc.tile_pool(name="dram", bufs=1, space="DRAM"))
    adj_pool = ctx.enter_context(tc.tile_pool(name="adj", bufs=2))
    t_pool = ctx.enter_context(tc.tile_pool(name="t", bufs=2))
    q_pool = ctx.enter_context(tc.tile_pool(name="q", bufs=2))
    fin_pool = ctx.enter_context(tc.tile_pool(name="fin", bufs=2))

    ident = const_pool.tile([P, P], f32, name="ident")
    masks.make_identity(nc, ident[:])
    ident_bf = const_pool.tile([P, P], bf16, name="ident_bf")
    masks.make_identity(nc, ident_bf[:])

    # ---- Phase 1: full hT = (x @ W).T and av = [a_s; a_n] for all nodes ----
    w_sb = []
    for k in range(kc_n):
        wt = ph1_pool.tile([P, dout], bf16, name="w_sb", tag=f"w_sb{k}")
        nc.sync.dma_start(wt[:], wmat[k * P:(k + 1) * P, :])
        w_sb.append(wt)
    att_sb = const_pool.tile([P, 2], bf16, name="att_sb")
    nc.sync.dma_start(att_sb[:], att[:])

    hT_sb = ph1_pool.tile([P, n], bf16, name="hT_sb")
    av_sb = ph1_pool.tile([2, n], f32, name="av_sb")  # rows: a_s, a_n
    for half in range(n // XT):
        xh = []
        for k in range(kc_n):
            xk = x_pool.tile([P, XT], bf16, name="x_sb")
            nc.sync.dma_start(xk[:], xt[k * P:(k + 1) * P, half * XT:(half + 1) * XT])
            xh.append(xk)
        for bb in range(XT // nb):
            b0 = half * XT + bb * nb
            hT_ps = tp_psum.tile([P, nb], f32, name="hT_ps", tag="tp")
            for k in range(kc_n):
                nc.tensor.matmul(
                    hT_ps[:], w_sb[k][:], xh[k][:, bb * nb:(bb + 1) * nb],
                    start=(k == 0), stop=(k == kc_n - 1),
                )
            nc.scalar.activation(hT_sb[:, b0:b0 + nb], hT_ps[:], AF.Copy)
            av_ps = tp_psum.tile([2, nb], f32, name="av_ps", tag="tp")
            nc.tensor.matmul(
                av_ps[:], att_sb[:], hT_sb[:, b0:b0 + nb],
                start=True, stop=True,
            )
            nc.scalar.activation(av_sb[:, b0:b0 + nb], av_ps[:], AF.Copy)

    # ---- Phase 2a: per-chunk scalars v = e^{a_n}, v2 = e^{0.2 a_n} ---------
    # a_n row -> DRAM -> [64, 128] -> transpose -> [128(p), 64(chunk)]
    an_dram = dram_pool.tile([n, 1], f32, name="an_dram")
    nc.sync.dma_start(an_dram[:].rearrange("s o -> o s"), av_sb[1:2, :])
    anf_raw = ph1_pool.tile([jc_n, P], f32, name="anf_raw")
    nc.sync.dma_start(anf_raw[:], an_dram[:].rearrange("(k p) o -> k (p o)", p=P))
    anf_ps = tp_psum.tile([P, jc_n], f32, name="anf_ps", tag="tp")
    nc.tensor.matmul(anf_ps[:], anf_raw[:], ident[:jc_n, :jc_n],
                     is_transpose=True, start=True, stop=True)
    vf_sb = const_pool.tile([P, jc_n], f32, name="vf_sb")
    nc.scalar.activation(vf_sb[:], anf_ps[:], AF.Exp, scale=1.0)
    v2f_sb = const_pool.tile([P, jc_n], f32, name="v2f_sb")
    nc.scalar.activation(v2f_sb[:], anf_ps[:], AF.Exp, scale=0.2)

    # wb[p, i] = exp(0.8 * a_s_local[i]) broadcast to all partitions (bf16).
    # Host rotation puts this core's slab at nodes [0, s).
    wrow_sb = ph1_pool.tile([1, s], f32, name="wrow_sb")
    nc.scalar.activation(wrow_sb[:], av_sb[0:1, :s], AF.Exp, scale=0.8)
    ones_sb = const_pool.tile([1, P], f32, name="ones_sb")
    nc.gpsimd.memset(ones_sb[:], 1.0)
    wb_sb = const_pool.tile([P, s], bf16, name="wb_sb")
    for b in range(s // nb):
        wb_ps = tp_psum.tile([P, nb], f32, name="wb_ps", tag="tp")
        nc.tensor.matmul(
            wb_ps[:], ones_sb[:], wrow_sb[:, b * nb:(b + 1) * nb],
            start=True, stop=True,
        )
        nc.scalar.activation(wb_sb[:, b * nb:(b + 1) * nb], wb_ps[:], AF.Copy)

    # ---- Phase 2b: h3big[p, j*130 + d] = h[j*128+p, d]; col 128 = 1 --------
    h3big = ph1_pool.tile([P, jc_n * dp], bf16, name="h3big")
    for c4 in range(jc_n // 4):
        tr_ps = tp_psum.tile([P, 4 * P], bf16, name="tr_ps", tag="tp")
        for u in range(4):
            c = c4 * 4 + u
            nc.tensor.matmul(
                tr_ps[:, u * P:(u + 1) * P], hT_sb[:, c * P:(c + 1) * P],
                ident_bf[:], is_transpose=True, start=True, stop=True,
            )
        # strided dest: 4 chunks of 128 cols at stride 130
        dst = h3big[:, c4 * 4 * dp:(c4 + 1) * 4 * dp].rearrange(
            "p (c d) -> p c d", c=4, d=dp)[:, :, :dout]
        nc.scalar.activation(dst, tr_ps[:].rearrange("p (c d) -> p c d", c=4),
                             AF.Copy)
    # ones in col 128 (denominator source); col 129 also set (pad, unused)
    onecol = h3big[:].rearrange("p (c d) -> p c d", c=jc_n)[:, :, dout:dp]
    nc.gpsimd.memset(onecol, 1.0)

    if DEBUG:
        dbg_hT = nc.dram_tensor("dbg_hT", [P, n], bf16, kind="ExternalOutput")
        nc.sync.dma_start(dbg_hT[:], hT_sb[:])
        dbg_av = nc.dram_tensor("dbg_av", [2, n], f32, kind="ExternalOutput")
        nc.sync.dma_start(dbg_av[:], av_sb[:])
        dbg_wb = nc.dram_tensor("dbg_wb", [P, s], bf16, kind="ExternalOutput")
        nc.sync.dma_start(dbg_wb[:], wb_sb[:])
        dbg_vf = nc.dram_tensor("dbg_vf", [P, jc_n], f32, kind="ExternalOutput")
        nc.sync.dma_start(dbg_vf[:], vf_sb[:])
        dbg_v2f = nc.dram_tensor("dbg_v2f", [P, jc_n], f32, kind="ExternalOutput")
        nc.sync.dma_start(dbg_v2f[:], v2f_sb[:])
        dbg_h3 = nc.dram_tensor("dbg_h3", [P, jc_n * dp], bf16,
                                kind="ExternalOutput")
        nc.sync.dma_start(dbg_h3[:], h3big[:])
        dbg_t = nc.dram_tensor("dbg_t", [P, MG * s], bf16, kind="ExternalOutput")
        dbg_q = nc.dram_tensor("dbg_q", [P, MG * s], bf16, kind="ExternalOutput")

    # ---- Phase 3: main loop over merge groups ------------------------------
    # 4 full-bank psum tiles, each holds two [i_blk, 130] accumulators.
    # NOTE: a matmul's start=True clears has_written flags for its whole PSUM
    # bank, so only the FIRST slice's first matmul may use start=True; the
    # second slice's first matmul relies on cleared flags -> overwrite.
    mm_ps = [acc_psum.tile([P, 512], f32, name=f"mm_ps{v}") for v in range(4)]

    def acc_slice(b):
        return mm_ps[b // 2][:, (b % 2) * dp:(b % 2) * dp + dp]

    for G in range(mg_n):
        adj_t = adj_pool.tile([P, MG * s], bf16, name="adj_t")
        for half in range(MG // GP):
            g = G * (MG // GP) + half
            nc.sync.dma_start(
                adj_t[:, half * GP * s:(half + 1) * GP * s],
                adjt[g * GP * P:(g + 1) * GP * P, :].rearrange(
                    "(p r) i -> p (r i)", r=GP),
            )
        t_t = t_pool.tile([P, MG * s], bf16, name="t_t")
        for r in range(MG):
            j = G * MG + r
            # t = max(v_j * w_i, v2_j): per-partition AP scalars, 4x mode
            nc.vector.tensor_scalar(
                t_t[:, r * s:(r + 1) * s], wb_sb[:],
                vf_sb[:, j:j + 1], v2f_sb[:, j:j + 1], ALU.mult, ALU.max,
            )
        q_t = q_pool.tile([P, MG * s], bf16, name="q_t")
        nc.vector.tensor_tensor(q_t[:], t_t[:], adj_t[:], ALU.mult)
        if DEBUG and G == 0:
            nc.sync.dma_start(dbg_t[:], t_t[:])
            nc.sync.dma_start(dbg_q[:], q_t[:])
        for r in range(MG):
            j = G * MG + r
            rhs = h3big[:, j * dp:(j + 1) * dp]
            for b in range(ib_n):
                nc.tensor.matmul(
                    acc_slice(b),
                    q_t[:, r * s + b * P:r * s + (b + 1) * P],
                    rhs,
                    start=(j == 0 and b % 2 == 0), stop=(j == jc_n - 1),
                    skip_group_check=True,
                )

    # ---- Phase 4: normalize + relu, direct [i, d] layout -------------------
    for b in range(ib_n):
        ps = acc_slice(b)
        rr_sb = fin_pool.tile([P, 1], f32, name="rr_sb", tag="rr")
        nc.vector.reciprocal(rr_sb[:], ps[:, dout:dout + 1])
        oc_sb = fin_pool.tile([P, dout], f32, name="oc_sb")
        nc.scalar.activation(oc_sb[:], ps[:, :dout], AF.Relu, scale=rr_sb[:])
        nc.sync.dma_start(out[b * P:(b + 1) * P, :], oc_sb[:])


def build_nc(n=N, s=S, din=DIN, dout=DOUT):
    from contextlib import ExitStack

    import concourse.bacc as bacc
    import concourse.tile as tile

    nc = bacc.Bacc(
        "TRN2",
        target_bir_lowering=False,
        debug=False,
        num_devices=NCORES,
    )
    with tile.TileContext(nc) as tc, ExitStack() as ctx:
        _emit(nc, tc, ctx, n, s, din, dout)
    nc.compile()
    return nc


def prep_adjt(adj_slab_T):
    """[n, s] rotated adjacency (transposed) -> bf16 with GP-row interleave."""
    import ml_dtypes

    n, s = adj_slab_T.shape
    P = 128
    g = n // (GP * P)
    a = adj_slab_T.reshape(g, GP, P, s).transpose(0, 2, 1, 3).reshape(n, s)
    return np.ascontiguousarray(a.astype(ml_dtypes.bfloat16))


def make_in_maps(x, adj, W, attn_self, attn_neigh, s=S):
    import ml_dtypes

    att = np.concatenate([attn_self, attn_neigh], axis=1).astype(
        ml_dtypes.bfloat16)
    W16 = np.ascontiguousarray(W.astype(ml_dtypes.bfloat16))
    xT16 = np.ascontiguousarray(x.T.astype(ml_dtypes.bfloat16))  # [din, n]
    in_maps = []
    for c in range(NCORES):
        o = c * s
        # rotate node axis so this core's slab comes first
        xt_c = np.ascontiguousarray(np.roll(xT16, -o, axis=1))
        adjT_c = np.roll(adj[o:o + s, :].T, -o, axis=0)  # [n(rot), s]
        in_maps.append({
            "adjt": prep_adjt(adjT_c),
            "xt": xt_c,
            "wmat": W16,
            "att": att,
        })
    return in_maps


def kernel(x, adj, W, attn_self, attn_neigh):
    from concourse.bass_utils import run_bass_kernel_spmd

    x = np.asarray(x, dtype=np.float32)
    adj = np.asarray(adj, dtype=np.float32)
    W = np.asarray(W, dtype=np.float32)
    attn_self = np.asarray(attn_self, dtype=np.float32)
    attn_neigh = np.asarray(attn_neigh, dtype=np.float32)

    nc = build_nc()
    in_maps = make_in_maps(x, adj, W, attn_self, attn_neigh)
    res = run_bass_kernel_spmd(nc, in_maps, list(range(NCORES)))
    return np.concatenate([res.results[c]["out"] for c in range(NCORES)], axis=0)


# revision 20
# speedup vs baseline: 1.8096x; 1.2444x over previous
"""AttentiveGraphConvolution (GAT-style layer) on 8 trn2 NeuronCores.

Math (reference):
    h   = x @ W                       [N, D]
    a_s = h @ attn_self               [N, 1]
    a_n = h @ attn_neigh              [N, 1]
    e   = leaky_relu(a_s + a_n.T, 0.2)
    e   = e + NEG_INF * (1 - adj)
    out = relu(softmax(e, -1) @ h)

Reformulation (exact up to rounding), with s_ij = a_s[i] + a_n[j]:
    exp(leaky(s)) = exp(0.2 s) * max(exp(0.8 s), 1)
                  = u2_i * max(w_i * v_j, v2_j)
      w = e^{0.8 a_s},  v = e^{a_n},  v2 = e^{0.2 a_n}  (u2_i cancels in softmax)
    out_i = relu( (sum_j q_ji h3_j[:D]) / (sum_j q_ji h3_j[D]) )
      q_ji = adjT_ji * max(w_i * v_j, v2_j)     [j, i] layout (v2 folded in)
      h3_j = [h_j | 1]                          [j, D+1]  (denominator folded in)

Device work per 8-chunk group (1024 j's) per core:
    t_ji = max(w_i * v_j, v2_j)   8x DVE tensor_scalar (4x bf16 mode,
                                  per-partition AP scalars)
    q    = t * adjT               1x DVE tensor_tensor (2x bf16 mode)
    64x matmul: psum[i_blk, 0:130] += q[:, blk].T @ h3[j-chunk]  (q stationary)
The 130-wide moving operand folds the softmax denominator into the matmul
stream (col 128 = ones, col 129 = pad for 4-byte alignment); the output
lands directly in [i, d] layout; there is no second rs matmul pass.

NO COLLECTIVES: each core loads the full x (bf16, 8.4 MB) and computes the
full h3 locally -- profiling showed AllGather rendezvous + protocol cost
~70 us, far more than the extra DMA. Inputs are HOST-ROTATED per core
(node axis rolled so the core's own slab is block 0), which keeps the
program rank-independent: a_s is always read from columns [0, S).
"""

import numpy as np

N = 8192
DIN = 512
DOUT = 128
NCORES = 8
S = N // NCORES   # 1024 output rows per core
GP = 4            # adjacency j-chunks per DMA descriptor group (8 KB)
MG = 8            # j-chunks per merge group (DVE op granularity)
DEBUG = False     # add intermediate-dump outputs (set before build_nc)


def _emit(nc, tc, ctx, n, s, din, dout):
    from concourse import masks, mybir

    f32 = mybir.dt.float32
    bf16 = mybir.dt.bfloat16
    AF = mybir.ActivationFunctionType
    ALU = mybir.AluOpType

    P = 128
    jc_n = n // P        # 64 j chunks over all nodes
    kc_n = din // P      # 4 contraction chunks for x @ W
    nb = 512             # moving-dim block for phase-1 matmuls
    mg_n = jc_n // MG    # 8 merge groups
    ib_n = s // P        # 8 output row blocks
    dp = dout + 2        # h3 stride: [h | 1 | pad] -> 130 (4-byte aligned)
    XT = 2048            # x tile width (4 KB descriptors)

    adjt = nc.dram_tensor("adjt", [n, s], bf16, kind="ExternalInput")
    xt = nc.dram_tensor("xt", [din, n], bf16, kind="ExternalInput")
    wmat = nc.dram_tensor("wmat", [din, dout], bf16, kind="ExternalInput")
    att = nc.dram_tensor("att", [dout, 2], bf16, kind="ExternalInput")
    out = nc.dram_tensor("out", [s, dout], f32, kind="ExternalOutput")

    const_pool = ctx.enter_context(tc.tile_pool(name="const", bufs=1))
    ph1_pool = ctx.enter_context(tc.tile_pool(name="ph1", bufs=1))
    x_pool = ctx.enter_context(tc.tile_pool(name="x", bufs=5))
    tp_psum = ctx.enter_context(tc.tile_pool(name="tp_psum", bufs=3, space="PSUM"))
    acc_psum = ctx.enter_context(tc.tile_pool(name="acc_psum", bufs=1, space="PSUM"))
    dram_pool = ctx.enter_context(tc.tile_pool(name="dram", bufs=1, space="DRAM"))
    adj_pool = ctx.enter_context(tc.tile_pool(name="adj", bufs=2))
    t_pool = ctx.enter_context(tc.tile_pool(name="t", bufs=2))
    q_pool = ctx.enter_context(tc.tile_pool(name="q", bufs=2))
    fin_pool = ctx.enter_context(tc.tile_pool(name="fin", bufs=2))

    ident = const_pool.tile([P, P], f32, name="ident")
    masks.make_identity(nc, ident[:])
    ident_bf = const_pool.tile([P, P], bf16, name="ident_bf")
    masks.make_identity(nc, ident_bf[:])

    # ---- Phase 1: full hT = (x @ W).T and av = [a_s; a_n] for all nodes ----
    w_sb = []
    for k in range(kc_n):
        wt = ph1_pool.tile([P, dout], bf16, name="w_sb", tag=f"w_sb{k}")
        nc.sync.dma_start(wt[:], wmat[k * P:(k + 1) * P, :])
        w_sb.append(wt)
    att_sb = const_pool.tile([P, 2], bf16, name="att_sb")
    nc.sync.dma_start(att_sb[:], att[:])

    hT_sb = ph1_pool.tile([P, n], bf16, name="hT_sb")
    av_sb = ph1_pool.tile([2, n], f32, name="av_sb")  # rows: a_s, a_n
    an_dram = dram_pool.tile([n, 1], f32, name="an_dram")
    vf_sb = const_pool.tile([P, jc_n], f32, name="vf_sb")
    v2f_sb = const_pool.tile([P, jc_n], f32, name="v2f_sb")
    wrow_sb = ph1_pool.tile([1, s], f32, name="wrow_sb")
    wb_sb = const_pool.tile([P, s], bf16, name="wb_sb")
    h3big = ph1_pool.tile([P, jc_n * dp], bf16, name="h3big")
    ones_sb = const_pool.tile([1, P], f32, name="ones_sb")
    nc.gpsimd.memset(ones_sb[:], 1.0)

    # One 2048-node quarter at a time: x DMA -> hT -> av -> a_n roundtrip ->
    # vf/v2f cols -> h3big chunks. The merge pipeline (DVE) for group G only
    # needs quarter G/2's scalars + wb, so it starts ~15us in, overlapping
    # the rest of phase 1.
    qn = n // XT               # 4 quarters
    cq = XT // P               # 16 j-chunks per quarter
    for qt in range(qn):
        xh = []
        for k in range(kc_n):
            xk = x_pool.tile([P, XT], bf16, name="x_sb")
            nc.sync.dma_start(xk[:], xt[k * P:(k + 1) * P, qt * XT:(qt + 1) * XT])
            xh.append(xk)
        for bb in range(XT // nb):
            b0 = qt * XT + bb * nb
            hT_ps = tp_psum.tile([P, nb], f32, name="hT_ps", tag="tp")
            for k in range(kc_n):
                nc.tensor.matmul(
                    hT_ps[:], w_sb[k][:], xh[k][:, bb * nb:(bb + 1) * nb],
                    start=(k == 0), stop=(k == kc_n - 1),
                )
            nc.scalar.activation(hT_sb[:, b0:b0 + nb], hT_ps[:], AF.Copy)
            av_ps = tp_psum.tile([2, nb], f32, name="av_ps", tag="tp")
            nc.tensor.matmul(
                av_ps[:], att_sb[:], hT_sb[:, b0:b0 + nb],
                start=True, stop=True,
            )
            nc.scalar.activation(av_sb[:, b0:b0 + nb], av_ps[:], AF.Copy)

        # a_n roundtrip for this quarter -> vf/v2f columns
        sl = slice(qt * XT, (qt + 1) * XT)
        csl = slice(qt * cq, (qt + 1) * cq)
        nc.sync.dma_start(an_dram[sl].rearrange("s o -> o s"), av_sb[1:2, sl])
        anf_raw = ph1_pool.tile([cq, P], f32, name="anf_raw", tag=f"anf{qt}")
        nc.sync.dma_start(
            anf_raw[:],
            an_dram[sl].rearrange("(k p) o -> k (p o)", p=P))
        anf_ps = tp_psum.tile([P, cq], f32, name="anf_ps", tag="tp")
        nc.tensor.matmul(anf_ps[:], anf_raw[:], ident[:cq, :cq],
                         is_transpose=True, start=True, stop=True)
        nc.scalar.activation(vf_sb[:, csl], anf_ps[:], AF.Exp, scale=1.0)
        nc.scalar.activation(v2f_sb[:, csl], anf_ps[:], AF.Exp, scale=0.2)

        if qt == 0:
            # wb[p, i] = exp(0.8 * a_s_local[i]); host rotation puts the
            # local slab at nodes [0, s)
            nc.scalar.activation(wrow_sb[:], av_sb[0:1, :s], AF.Exp, scale=0.8)
            for b in range(s // nb):
                wb_ps = tp_psum.tile([P, nb], f32, name="wb_ps", tag="tp")
                nc.tensor.matmul(
                    wb_ps[:], ones_sb[:], wrow_sb[:, b * nb:(b + 1) * nb],
                    start=True, stop=True,
                )
                nc.scalar.activation(wb_sb[:, b * nb:(b + 1) * nb], wb_ps[:],
                                     AF.Copy)

        # h3big chunks for this quarter: h3big[p, c*130+d] = h[c*128+p, d]
        for c4 in range(qt * (cq // 4), (qt + 1) * (cq // 4)):
            tr_ps = tp_psum.tile([P, 4 * P], bf16, name="tr_ps", tag="tp")
            for u in range(4):
                c = c4 * 4 + u
                nc.tensor.matmul(
                    tr_ps[:, u * P:(u + 1) * P], hT_sb[:, c * P:(c + 1) * P],
                    ident_bf[:], is_transpose=True, start=True, stop=True,
                )
            dst = h3big[:, c4 * 4 * dp:(c4 + 1) * 4 * dp].rearrange(
                "p (c d) -> p c d", c=4, d=dp)[:, :, :dout]
            nc.scalar.activation(
                dst, tr_ps[:].rearrange("p (c d) -> p c d", c=4), AF.Copy)
        # ones in col 128 (denominator source); col 129 also set (pad)
        onecol = h3big[:, qt * cq * dp:(qt + 1) * cq * dp].rearrange(
            "p (c d) -> p c d", c=cq)[:, :, dout:dp]
        nc.gpsimd.memset(onecol, 1.0)

    if DEBUG:
        dbg_hT = nc.dram_tensor("dbg_hT", [P, n], bf16, kind="ExternalOutput")
        nc.sync.dma_start(dbg_hT[:], hT_sb[:])
        dbg_av = nc.dram_tensor("dbg_av", [2, n], f32, kind="ExternalOutput")
        nc.sync.dma_start(dbg_av[:], av_sb[:])
        dbg_wb = nc.dram_tensor("dbg_wb", [P, s], bf16, kind="ExternalOutput")
        nc.sync.dma_start(dbg_wb[:], wb_sb[:])
        dbg_vf = nc.dram_tensor("dbg_vf", [P, jc_n], f32, kind="ExternalOutput")
        nc.sync.dma_start(dbg_vf[:], vf_sb[:])
        dbg_v2f = nc.dram_tensor("dbg_v2f", [P, jc_n], f32, kind="ExternalOutput")
        nc.sync.dma_start(dbg_v2f[:], v2f_sb[:])
        dbg_h3 = nc.dram_tensor("dbg_h3", [P, jc_n * dp], bf16,
                                kind="ExternalOutput")
        nc.sync.dma_start(dbg_h3[:], h3big[:])
        dbg_t = nc.dram_tensor("dbg_t", [P, MG * s], bf16, kind="ExternalOutput")
        dbg_q = nc.dram_tensor("dbg_q", [P, MG * s], bf16, kind="ExternalOutput")

    # ---- Phase 3: main loop over merge groups ------------------------------
    # 4 full-bank psum tiles, each holds two [i_blk, 130] accumulators.
    # NOTE: a matmul's start=True clears has_written flags for its whole PSUM
    # bank, so only the FIRST slice's first matmul may use start=True; the
    # second slice's first matmul relies on cleared flags -> overwrite.
    mm_ps = [acc_psum.tile([P, 512], f32, name=f"mm_ps{v}") for v in range(4)]

    def acc_slice(b):
        return mm_ps[b // 2][:, (b % 2) * dp:(b % 2) * dp + dp]

    for G in range(mg_n):
        adj_t = adj_pool.tile([P, MG * s], bf16, name="adj_t")
        for half in range(MG // GP):
            g = G * (MG // GP) + half
            nc.sync.dma_start(
                adj_t[:, half * GP * s:(half + 1) * GP * s],
                adjt[g * GP * P:(g + 1) * GP * P, :].rearrange(
                    "(p r) i -> p (r i)", r=GP),
            )
        t_t = t_pool.tile([P, MG * s], bf16, name="t_t")
        for r in range(MG):
            j = G * MG + r
            # t = max(v_j * w_i, v2_j): per-partition AP scalars, 4x mode
            nc.vector.tensor_scalar(
                t_t[:, r * s:(r + 1) * s], wb_sb[:],
                vf_sb[:, j:j + 1], v2f_sb[:, j:j + 1], ALU.mult, ALU.max,
            )
        q_t = q_pool.tile([P, MG * s], bf16, name="q_t")
        if G < mg_n - 1:
            nc.vector.tensor_tensor(q_t[:], t_t[:], adj_t[:], ALU.mult)
        else:
            # split the final merge so the PE drain tail is short
            for r0, rl in ((0, 4), (4, 3), (7, 1)):
                sl2 = slice(r0 * s, (r0 + rl) * s)
                nc.vector.tensor_tensor(q_t[:, sl2], t_t[:, sl2],
                                        adj_t[:, sl2], ALU.mult)
        if DEBUG and G == 0:
            nc.sync.dma_start(dbg_t[:], t_t[:])
            nc.sync.dma_start(dbg_q[:], q_t[:])
        for r in range(MG):
            j = G * MG + r
            rhs = h3big[:, j * dp:(j + 1) * dp]
            for b in range(ib_n):
                nc.tensor.matmul(
                    acc_slice(b),
                    q_t[:, r * s + b * P:r * s + (b + 1) * P],
                    rhs,
                    start=(j == 0 and b % 2 == 0), stop=(j == jc_n - 1),
                    skip_group_check=True,
                )

    # ---- Phase 4: normalize + relu, direct [i, d] layout -------------------
    for b in range(ib_n):
        ps = acc_slice(b)
        rr_sb = fin_pool.tile([P, 1], f32, name="rr_sb", tag="rr")
        nc.vector.reciprocal(rr_sb[:], ps[:, dout:dout + 1])
        oc_sb = fin_pool.tile([P, dout], f32, name="oc_sb")
        nc.scalar.activation(oc_sb[:], ps[:, :dout], AF.Relu, scale=rr_sb[:])
        nc.sync.dma_start(out[b * P:(b + 1) * P, :], oc_sb[:])


def build_nc(n=N, s=S, din=DIN, dout=DOUT):
    from contextlib import ExitStack

    import concourse.bacc as bacc
    import concourse.tile as tile

    nc = bacc.Bacc(
        "TRN2",
        target_bir_lowering=False,
        debug=False,
        num_devices=NCORES,
    )
    with tile.TileContext(nc) as tc, ExitStack() as ctx:
        _emit(nc, tc, ctx, n, s, din, dout)
    nc.compile()
    return nc


def prep_adjt(adj_slab_T):
    """[n, s] rotated adjacency (transposed) -> bf16 with GP-row interleave."""
    import ml_dtypes

    n, s = adj_slab_T.shape
    P = 128
    g = n // (GP * P)
    a = adj_slab_T.reshape(g, GP, P, s).transpose(0, 2, 1, 3).reshape(n, s)
    return np.ascontiguousarray(a.astype(ml_dtypes.bfloat16))


def make_in_maps(x, adj, W, attn_self, attn_neigh, s=S):
    import ml_dtypes

    att = np.concatenate([attn_self, attn_neigh], axis=1).astype(
        ml_dtypes.bfloat16)
    W16 = np.ascontiguousarray(W.astype(ml_dtypes.bfloat16))
    xT16 = np.ascontiguousarray(x.T.astype(ml_dtypes.bfloat16))  # [din, n]
    in_maps = []
    for c in range(NCORES):
        o = c * s
        # rotate node axis so this core's slab comes first
        xt_c = np.ascontiguousarray(np.roll(xT16, -o, axis=1))
        adjT_c = np.roll(adj[o:o + s, :].T, -o, axis=0)  # [n(rot), s]
        in_maps.append({
            "adjt": prep_adjt(adjT_c),
            "xt": xt_c,
            "wmat": W16,
            "att": att,
        })
    return in_maps


def kernel(x, adj, W, attn_self, attn_neigh):
    from concourse.bass_utils import run_bass_kernel_spmd

    x = np.asarray(x, dtype=np.float32)
    adj = np.asarray(adj, dtype=np.float32)
    W = np.asarray(W, dtype=np.float32)
    attn_self = np.asarray(attn_self, dtype=np.float32)
    attn_neigh = np.asarray(attn_neigh, dtype=np.float32)

    nc = build_nc()
    in_maps = make_in_maps(x, adj, W, attn_self, attn_neigh)
    res = run_bass_kernel_spmd(nc, in_maps, list(range(NCORES)))
    return np.concatenate([res.results[c]["out"] for c in range(NCORES)], axis=0)
